# revision 6
# baseline (speedup 1.0000x reference)
"""Distributed Bass attention kernel for 8 TRN2 NeuronCores.

Problem: single-head causal attention, B=4, S=2048, d_model=1024, d_head=64.
  q = x@WQ.T+bq; k = x@WK.T+bk; v = x@WV.T+bv (v is d_model wide)
  out = softmax(causal(q@k.T)) @ v

Sharding: core = 2*b + half. Each core computes batch b, output channels
[half*512, (half+1)*512). Q/K/scores/softmax are duplicated within a batch
pair (cheap); V projection and attn@V are channel-split. No collectives.

Layout tricks:
  - x fed pre-transposed (xT [d, S]) so projections contract d on partitions.
  - scores computed transposed [keys, queries] (kT stationary, qT moving) so
    attn@V uses P tiles directly as the stationary operand - no transposes.
  - softmax without max-subtraction (|logits| <= ~45 => exp fits fp32 fine);
    rowsum via an extra N=1 matmul vs a ones vector; normalize at the end
    with DVE reciprocal + per-partition scalar multiply.
  - float32r (1 PE cycle/row vs 4 for fp32) for the q/k path (logit
    precision); bf16 for the v path and P (post-softmax quantization only).
"""

import sys

if "/opt/trn_rl_repo" not in sys.path:
    sys.path.insert(0, "/opt/trn_rl_repo")

import numpy as np

from concourse import bacc, tile, mybir
import concourse.bass as bass
from concourse.bass_utils import run_bass_kernel_spmd

B, S, D, HD = 4, 2048, 1024, 64
N_CORES = 8
CPC = 512  # output channels per core
NCHUNK = 8  # d_model / 128

f32 = mybir.dt.float32
f32r = mybir.dt.float32r
bf16 = mybir.dt.bfloat16
AF = mybir.ActivationFunctionType
ALU = mybir.AluOpType

_cache = {}


def _build():
    nc = bacc.Bacc("TRN2", target_bir_lowering=False, debug=False, num_devices=N_CORES)

    xT = nc.dram_tensor("xT", [NCHUNK, 128, S], f32r, kind="ExternalInput")
    wqkT = nc.dram_tensor("wqkT", [NCHUNK, 128, 128], f32r, kind="ExternalInput")
    bqk = nc.dram_tensor("bqk", [128, 1], f32, kind="ExternalInput")
    wvT = nc.dram_tensor("wvT", [NCHUNK, 128, CPC], bf16, kind="ExternalInput")
    bv = nc.dram_tensor("bv", [1, CPC], f32r, kind="ExternalInput")
    masks = nc.dram_tensor("masks", [4, 128, 512], bf16, kind="ExternalInput")
    ones1 = nc.dram_tensor("ones1", [1, 128], f32r, kind="ExternalInput")
    out = nc.dram_tensor("out", [16, 128, CPC], f32, kind="ExternalOutput")

    with tile.TileContext(nc) as tc:
        with (
            tc.tile_pool(name="big", bufs=1) as big,
            tc.tile_pool(name="ppool", bufs=20) as ppool,
            tc.tile_pool(name="opool", bufs=3) as opool,
            tc.tile_pool(name="small", bufs=4) as small,
            tc.tile_pool(name="ps_s", bufs=3, space=bass.MemorySpace.PSUM) as ps_s,
            tc.tile_pool(name="ps_v", bufs=2, space=bass.MemorySpace.PSUM) as ps_v,
            tc.tile_pool(name="ps_o", bufs=2, space=bass.MemorySpace.PSUM) as ps_o,
            tc.tile_pool(name="ps_r", bufs=1, space=bass.MemorySpace.PSUM) as ps_r,
        ):
            # persistent SBUF tiles
            xt = big.tile([128, NCHUNK, S], f32r, tag="xt")  # 64KB/p
            xt_bf = big.tile([128, NCHUNK, S], bf16, tag="xtbf")  # 32KB/p
            wqk = big.tile([128, NCHUNK, 128], f32r, tag="wqk")  # 4KB/p
            wv = big.tile([128, NCHUNK, CPC], bf16, tag="wv")  # 8KB/p
            bqk_sb = big.tile([128, 1], f32, tag="bqk")
            bv_sb = big.tile([1, CPC], f32r, tag="bv")
            mask_sb = big.tile([128, 4, 512], bf16, tag="mask")  # 4KB/p
            qk_sb = big.tile([128, S], f32r, tag="qk")  # 8KB/p
            kt_sb = big.tile([64, S], f32r, tag="kt")  # 8KB/p
            v_sb = big.tile([128, 16, CPC], bf16, tag="v")  # 16KB/p
            ones_k = big.tile([128, 1], bf16, tag="ones_k")
            ones_1 = big.tile([1, 128], f32r, tag="ones_1")

            # input DMAs, most-urgent first: wqk+xt feed the first matmuls
            for c in range(NCHUNK):
                nc.sync.dma_start(out=wqk[:, c, :], in_=wqkT[c, :, :])
                nc.sync.dma_start(out=xt[:, c, :], in_=xT[c, :, :])
            nc.sync.dma_start(out=bqk_sb[:, :], in_=bqk[:, :])
            for c in range(NCHUNK):
                nc.sync.dma_start(out=wv[:, c, :], in_=wvT[c, :, :])
            for m in range(4):
                nc.sync.dma_start(out=mask_sb[:, m, :], in_=masks[m, :, :])
            nc.sync.dma_start(out=bv_sb[:, :], in_=bv[:, :])
            nc.sync.dma_start(out=ones_1[:, :], in_=ones1[:, :])
            nc.vector.memset(ones_k[:, :], 1.0)

            # bf16 copy of xT for the V projection (GpSimd is otherwise idle)
            for c in range(NCHUNK):
                nc.gpsimd.tensor_copy(xt_bf[:, c, :], xt[:, c, :].bitcast(f32))

            # ---- Q/K projection: qkT [128h (64 q + 64 k), S] ----
            # chunk-outer (pairs of blocks) so compute starts on chunk 0;
            # accumulators share the scores pool's PSUM slots (same tag)
            for pair in range(2):
                qk_ps = [
                    ps_s.tile([128, 512], f32, tag="scps", name=f"qkps{pair}{jj}")
                    for jj in range(2)
                ]
                for c in range(NCHUNK):
                    for jj in range(2):
                        j = 2 * pair + jj
                        nc.tensor.matmul(
                            qk_ps[jj][:, :],
                            wqk[:, c, :],
                            xt[:, c, 512 * j : 512 * (j + 1)],
                            start=(c == 0),
                            stop=(c == NCHUNK - 1),
                        )
                for jj in range(2):
                    j = 2 * pair + jj
                    nc.scalar.activation(
                        qk_sb[:, 512 * j : 512 * (j + 1)],
                        qk_ps[jj][:, :],
                        AF.Identity,
                        bias=bqk_sb[:, 0:1],
                    )
                    # kT rows (64..128) -> partitions 0..64, per block
                    nc.sync.dma_start(
                        out=kt_sb[:, 512 * j : 512 * (j + 1)],
                        in_=qk_sb[64:128, 512 * j : 512 * (j + 1)],
                    )

            # ---- V projection (bf16; bias folded via K=1 ones matmul) ----
            for t in range(16):
                v_ps = ps_v.tile([128, CPC], f32, tag="vps")
                for c in range(NCHUNK):
                    nc.tensor.matmul(
                        v_ps[:, :],
                        xt_bf[:, c, 128 * t : 128 * (t + 1)],
                        wv[:, c, :],
                        start=(c == 0),
                        stop=False,
                    )
                nc.tensor.matmul(
                    v_ps[:, :], ones_1[:, :], bv_sb[:, :], start=False, stop=True
                )
                nc.scalar.copy(v_sb[:, t, :], v_ps[:, :])

            # ---- attention ----
            for j in range(4):  # query block of 512
                P = []
                for i in range(4 * j + 4):  # key tile of 128
                    sc_ps = ps_s.tile([128, 512], f32, tag="scps")
                    nc.tensor.matmul(
                        sc_ps[:, :],
                        kt_sb[:, 128 * i : 128 * (i + 1)],
                        qk_sb[0:64, 512 * j : 512 * (j + 1)],
                        start=True,
                        stop=True,
                    )
                    p = ppool.tile([128, 512], bf16, tag="p")
                    nc.scalar.activation(p[:, :], sc_ps[:, :], AF.Exp)
                    if i >= 4 * j:
                        nc.vector.tensor_tensor(
                            p[:, :], p[:, :], mask_sb[:, i - 4 * j, :], ALU.mult
                        )
                    P.append(p)
                for tq in range(4):  # query tile of 128 within the block
                    t = 4 * j + tq
                    o_ps = ps_o.tile([128, CPC], f32, tag="ops")
                    rs_ps = ps_r.tile([128, 1], f32, tag="rsps")
                    for i in range(t + 1):
                        lhsT = P[i][:, 128 * tq : 128 * (tq + 1)]
                        nc.tensor.matmul(
                            o_ps[:, :],
                            lhsT,
                            v_sb[:, i, :],
                            start=(i == 0),
                            stop=(i == t),
                        )
                        nc.tensor.matmul(
                            rs_ps[:, :],
                            lhsT,
                            ones_k[:, :],
                            start=(i == 0),
                            stop=(i == t),
                        )
                    rs_sb = small.tile([128, 1], f32, tag="rs")
                    nc.scalar.copy(rs_sb[:, :], rs_ps[:, :])
                    rcp = small.tile([128, 1], f32, tag="rcp")
                    nc.vector.reciprocal(rcp[:, :], rs_sb[:, :])
                    o_sb = opool.tile([128, CPC], f32, tag="osb")
                    nc.vector.tensor_scalar(
                        o_sb[:, :], o_ps[:, :], rcp[:, 0:1], None, ALU.mult
                    )
                    nc.sync.dma_start(out=out[t, :, :], in_=o_sb[:, :])

    nc.compile()
    return nc


def _get_nc():
    if "nc" not in _cache:
        _cache["nc"] = _build()
    return _cache["nc"]


def _prep_in_maps(x, WQ_w, WQ_b, WK_w, WK_b, WV_w, WV_b):
    bf = mybir.dt.np(bf16)
    wqk = np.concatenate([WQ_w, WK_w], axis=0)  # [128, D]
    wqkT = np.ascontiguousarray(wqk.T.reshape(NCHUNK, 128, 128)).astype(
        np.float32, copy=False
    )
    bqk = np.concatenate([WQ_b, WK_b]).reshape(128, 1).astype(np.float32, copy=False)

    # masks[m, kk, qq] = 1 if 128*m + kk <= qq else 0
    kk = np.arange(128)[:, None]
    qq = np.arange(512)[None, :]
    masks = np.stack([(128 * m + kk <= qq) for m in range(4)], axis=0).astype(bf)

    in_maps = []
    for core in range(N_CORES):
        b, half = core // 2, core % 2
        xTb = np.ascontiguousarray(x[b].T).reshape(NCHUNK, 128, S)
        wv_sl = WV_w[half * CPC : (half + 1) * CPC]  # [CPC, D]
        wvT = np.ascontiguousarray(wv_sl.T).reshape(NCHUNK, 128, CPC)
        bv = np.ascontiguousarray(WV_b[half * CPC : (half + 1) * CPC].reshape(1, CPC))
        in_maps.append(
            {
                "xT": xTb.astype(np.float32, copy=False),
                "wqkT": wqkT,
                "bqk": bqk,
                "wvT": wvT.astype(bf),
                "bv": bv.astype(np.float32, copy=False),
                "masks": masks,
                "ones1": np.ones((1, 128), np.float32),
            }
        )
    return in_maps


def _run(in_maps, trace=False, **kw):
    nc = _get_nc()
    return run_bass_kernel_spmd(
        nc, in_maps, core_ids=list(range(N_CORES)), trace=trace, **kw
    )


def kernel(x, WQ_w, WQ_b, WK_w, WK_b, WV_w, WV_b):
    x = np.asarray(x, dtype=np.float32)
    in_maps = _prep_in_maps(
        x,
        np.asarray(WQ_w, np.float32),
        np.asarray(WQ_b, np.float32),
        np.asarray(WK_w, np.float32),
        np.asarray(WK_b, np.float32),
        np.asarray(WV_w, np.float32),
        np.asarray(WV_b, np.float32),
    )
    res = _run(in_maps, trace=False)
    out = np.empty((B, S, D), dtype=np.float32)
    for core in range(N_CORES):
        b, half = core // 2, core % 2
        shard = res.results[core]["out"].reshape(S, CPC)
        out[b, :, half * CPC : (half + 1) * CPC] = shard
    return out


# revision 7
# speedup vs baseline: 1.1008x; 1.1008x over previous
"""Distributed Bass attention kernel for 8 TRN2 NeuronCores.

Problem: single-head causal attention, B=4, S=2048, d_model=1024, d_head=64.
  q = x@WQ.T+bq; k = x@WK.T+bk; v = x@WV.T+bv (v is d_model wide)
  out = softmax(causal(q@k.T)) @ v

Sharding: core = 2*b + half. Each core computes batch b, output channels
[half*512, (half+1)*512). Q/K/scores/softmax are duplicated within a batch
pair (cheap); V projection and attn@V are channel-split. No collectives.

Layout tricks:
  - x fed pre-transposed (xT [d, S]) so projections contract d on partitions.
  - scores computed transposed [keys, queries] (kT stationary, qT moving) so
    attn@V uses P tiles directly as the stationary operand - no transposes.
  - softmax without max-subtraction (|logits| <= ~45 => exp fits fp32 fine);
    rowsum via an extra N=1 matmul vs a ones vector; normalize at the end
    with DVE reciprocal + per-partition scalar multiply.
  - float32r (1 PE cycle/row vs 4 for fp32) for the q/k path (logit
    precision); bf16 for the v path and P (post-softmax quantization only).
"""

import sys

if "/opt/trn_rl_repo" not in sys.path:
    sys.path.insert(0, "/opt/trn_rl_repo")

import numpy as np

from concourse import bacc, tile, mybir
import concourse.bass as bass
from concourse.bass_utils import run_bass_kernel_spmd

B, S, D, HD = 4, 2048, 1024, 64
N_CORES = 8
CPC = 512  # output channels per core
NCHUNK = 8  # d_model / 128

f32 = mybir.dt.float32
f32r = mybir.dt.float32r
bf16 = mybir.dt.bfloat16
AF = mybir.ActivationFunctionType
ALU = mybir.AluOpType

_cache = {}


def _build():
    nc = bacc.Bacc("TRN2", target_bir_lowering=False, debug=False, num_devices=N_CORES)

    xT = nc.dram_tensor("xT", [NCHUNK, 128, S], bf16, kind="ExternalInput")
    wqkT = nc.dram_tensor("wqkT", [NCHUNK, 128, 128], bf16, kind="ExternalInput")
    bqk = nc.dram_tensor("bqk", [128, 1], f32, kind="ExternalInput")
    wvT = nc.dram_tensor("wvT", [NCHUNK, 128, CPC], bf16, kind="ExternalInput")
    bv = nc.dram_tensor("bv", [1, CPC], f32r, kind="ExternalInput")
    masks = nc.dram_tensor("masks", [4, 128, 512], bf16, kind="ExternalInput")
    ones1 = nc.dram_tensor("ones1", [1, 128], f32r, kind="ExternalInput")
    out = nc.dram_tensor("out", [16, 128, CPC], f32, kind="ExternalOutput")

    with tile.TileContext(nc) as tc:
        with (
            tc.tile_pool(name="big", bufs=1) as big,
            tc.tile_pool(name="ppool", bufs=20) as ppool,
            tc.tile_pool(name="opool", bufs=3) as opool,
            tc.tile_pool(name="small", bufs=4) as small,
            tc.tile_pool(name="ps_s", bufs=3, space=bass.MemorySpace.PSUM) as ps_s,
            tc.tile_pool(name="ps_v", bufs=2, space=bass.MemorySpace.PSUM) as ps_v,
            tc.tile_pool(name="ps_o", bufs=2, space=bass.MemorySpace.PSUM) as ps_o,
            tc.tile_pool(name="ps_rb", bufs=1, space=bass.MemorySpace.PSUM) as ps_rb,
        ):
            # persistent SBUF tiles
            xt = big.tile([128, NCHUNK, S], bf16, tag="xt")  # 32KB/p
            wqk = big.tile([128, NCHUNK, 128], bf16, tag="wqk")  # 2KB/p
            wv = big.tile([128, NCHUNK, CPC], bf16, tag="wv")  # 8KB/p
            bqk_sb = big.tile([128, 1], f32, tag="bqk")
            bv_sb = big.tile([1, CPC], f32r, tag="bv")
            mask_sb = big.tile([128, 4, 512], bf16, tag="mask")  # 4KB/p
            qk_sb = big.tile([128, S], f32r, tag="qk")  # 8KB/p
            kt_sb = big.tile([64, S], f32r, tag="kt")  # 8KB/p
            v_sb = big.tile([128, 16, CPC], bf16, tag="v")  # 16KB/p
            ones_b = big.tile([128, 128], bf16, tag="ones_b")
            ones_1 = big.tile([1, 128], f32r, tag="ones_1")

            # input DMAs, most-urgent first: wqk+xt feed the first matmuls
            for c in range(NCHUNK):
                nc.sync.dma_start(out=wqk[:, c, :], in_=wqkT[c, :, :])
                nc.sync.dma_start(out=xt[:, c, :], in_=xT[c, :, :])
            nc.sync.dma_start(out=bqk_sb[:, :], in_=bqk[:, :])
            for c in range(NCHUNK):
                nc.sync.dma_start(out=wv[:, c, :], in_=wvT[c, :, :])
            for m in range(4):
                nc.sync.dma_start(out=mask_sb[:, m, :], in_=masks[m, :, :])
            nc.sync.dma_start(out=bv_sb[:, :], in_=bv[:, :])
            nc.sync.dma_start(out=ones_1[:, :], in_=ones1[:, :])
            nc.vector.memset(ones_b[:, :], 1.0)

            # ---- Q/K projection: qkT [128h (64 q + 64 k), S] ----
            # chunk-outer (pairs of blocks) so compute starts on chunk 0;
            # accumulators share the scores pool's PSUM slots (same tag)
            for pair in range(2):
                qk_ps = [
                    ps_s.tile([128, 512], f32, tag="scps", name=f"qkps{pair}{jj}")
                    for jj in range(2)
                ]
                for c in range(NCHUNK):
                    for jj in range(2):
                        j = 2 * pair + jj
                        nc.tensor.matmul(
                            qk_ps[jj][:, :],
                            wqk[:, c, :],
                            xt[:, c, 512 * j : 512 * (j + 1)],
                            start=(c == 0),
                            stop=(c == NCHUNK - 1),
                        )
                for jj in range(2):
                    j = 2 * pair + jj
                    nc.scalar.activation(
                        qk_sb[:, 512 * j : 512 * (j + 1)],
                        qk_ps[jj][:, :],
                        AF.Identity,
                        bias=bqk_sb[:, 0:1],
                    )
                    # kT rows (64..128) -> partitions 0..64, per block
                    nc.sync.dma_start(
                        out=kt_sb[:, 512 * j : 512 * (j + 1)],
                        in_=qk_sb[64:128, 512 * j : 512 * (j + 1)],
                    )

            # ---- V projection (bf16; bias folded via K=1 ones matmul) ----
            for t in range(16):
                v_ps = ps_v.tile([128, CPC], f32, tag="vps")
                for c in range(NCHUNK):
                    nc.tensor.matmul(
                        v_ps[:, :],
                        xt[:, c, 128 * t : 128 * (t + 1)],
                        wv[:, c, :],
                        start=(c == 0),
                        stop=False,
                    )
                nc.tensor.matmul(
                    v_ps[:, :], ones_1[:, :], bv_sb[:, :], start=False, stop=True
                )
                nc.scalar.copy(v_sb[:, t, :], v_ps[:, :])

            # ---- attention ----
            for j in range(4):  # query block of 512
                P = []
                # scores -> exp -> mask, and accumulate the rowsum broadcast
                # (ones128.T @ P has every row equal to the key-sum per query)
                rb_ps = ps_rb.tile([128, 512], f32, tag="rbps", name=f"rbps{j}")
                for i in range(4 * j + 4):  # key tile of 128
                    sc_ps = ps_s.tile([128, 512], f32, tag="scps", name=f"scps{j}_{i}")
                    nc.tensor.matmul(
                        sc_ps[:, :],
                        kt_sb[:, 128 * i : 128 * (i + 1)],
                        qk_sb[0:64, 512 * j : 512 * (j + 1)],
                        start=True,
                        stop=True,
                    )
                    p = ppool.tile([128, 512], bf16, tag="p", name=f"p{j}_{i}")
                    nc.scalar.activation(p[:, :], sc_ps[:, :], AF.Exp)
                    if i >= 4 * j:
                        nc.vector.tensor_tensor(
                            p[:, :], p[:, :], mask_sb[:, i - 4 * j, :], ALU.mult
                        )
                    nc.tensor.matmul(
                        rb_ps[:, :],
                        ones_b[:, :],
                        p[:, :],
                        start=(i == 0),
                        stop=(i == 4 * j + 3),
                    )
                    P.append(p)
                # reciprocal of the broadcast rowsum, then prenormalize P
                rcp = small.tile([128, 512], f32, tag="rcp", name=f"rcp{j}")
                nc.vector.reciprocal(rcp[:, :], rb_ps[:, :])
                for i in range(4 * j + 4):
                    nc.vector.tensor_tensor(P[i][:, :], P[i][:, :], rcp[:, :], ALU.mult)
                for tq in range(4):  # query tile of 128 within the block
                    t = 4 * j + tq
                    o_ps = ps_o.tile([128, CPC], f32, tag="ops", name=f"ops{t}")
                    for i in range(t + 1):
                        nc.tensor.matmul(
                            o_ps[:, :],
                            P[i][:, 128 * tq : 128 * (tq + 1)],
                            v_sb[:, i, :],
                            start=(i == 0),
                            stop=(i == t),
                        )
                    o_sb = opool.tile([128, CPC], f32, tag="osb", name=f"osb{t}")
                    nc.scalar.copy(o_sb[:, :], o_ps[:, :])
                    nc.sync.dma_start(out=out[t, :, :], in_=o_sb[:, :])

    nc.compile()
    return nc


def _get_nc():
    if "nc" not in _cache:
        _cache["nc"] = _build()
    return _cache["nc"]


def _prep_in_maps(x, WQ_w, WQ_b, WK_w, WK_b, WV_w, WV_b):
    bf = mybir.dt.np(bf16)
    wqk = np.concatenate([WQ_w, WK_w], axis=0)  # [128, D]
    wqkT = np.ascontiguousarray(wqk.T.reshape(NCHUNK, 128, 128)).astype(bf)
    bqk = np.concatenate([WQ_b, WK_b]).reshape(128, 1).astype(np.float32, copy=False)

    # masks[m, kk, qq] = 1 if 128*m + kk <= qq else 0
    kk = np.arange(128)[:, None]
    qq = np.arange(512)[None, :]
    masks = np.stack([(128 * m + kk <= qq) for m in range(4)], axis=0).astype(bf)

    in_maps = []
    for core in range(N_CORES):
        b, half = core // 2, core % 2
        xTb = np.ascontiguousarray(x[b].T).reshape(NCHUNK, 128, S)
        wv_sl = WV_w[half * CPC : (half + 1) * CPC]  # [CPC, D]
        wvT = np.ascontiguousarray(wv_sl.T).reshape(NCHUNK, 128, CPC)
        bv = np.ascontiguousarray(WV_b[half * CPC : (half + 1) * CPC].reshape(1, CPC))
        in_maps.append(
            {
                "xT": xTb.astype(bf),
                "wqkT": wqkT,
                "bqk": bqk,
                "wvT": wvT.astype(bf),
                "bv": bv.astype(np.float32, copy=False),
                "masks": masks,
                "ones1": np.ones((1, 128), np.float32),
            }
        )
    return in_maps


def _run(in_maps, trace=False, **kw):
    nc = _get_nc()
    return run_bass_kernel_spmd(
        nc, in_maps, core_ids=list(range(N_CORES)), trace=trace, **kw
    )


def kernel(x, WQ_w, WQ_b, WK_w, WK_b, WV_w, WV_b):
    x = np.asarray(x, dtype=np.float32)
    in_maps = _prep_in_maps(
        x,
        np.asarray(WQ_w, np.float32),
        np.asarray(WQ_b, np.float32),
        np.asarray(WK_w, np.float32),
        np.asarray(WK_b, np.float32),
        np.asarray(WV_w, np.float32),
        np.asarray(WV_b, np.float32),
    )
    res = _run(in_maps, trace=False)
    out = np.empty((B, S, D), dtype=np.float32)
    for core in range(N_CORES):
        b, half = core // 2, core % 2
        shard = res.results[core]["out"].reshape(S, CPC)
        out[b, :, half * CPC : (half + 1) * CPC] = shard
    return out


# revision 9
# speedup vs baseline: 1.3293x; 1.2075x over previous
"""Distributed Bass attention kernel for 8 TRN2 NeuronCores.

Problem: single-head causal attention, B=4, S=2048, d_model=1024, d_head=64.
  q = x@WQ.T+bq; k = x@WK.T+bk; v = x@WV.T+bv (v is d_model wide)
  out = softmax(causal(q@k.T)) @ v

Sharding: core = 2*b + half. Each core computes batch b, output channels
[half*512, (half+1)*512). Q/K/scores/softmax are duplicated within a batch
pair (cheap); V projection and attn@V are channel-split. No collectives.

Layout tricks:
  - x fed pre-transposed (xT [d, S]) so projections contract d on partitions.
  - scores computed transposed [keys, queries] (kT stationary, qT moving) so
    attn@V uses P tiles directly as the stationary operand - no transposes.
  - softmax without max-subtraction (|logits| <= ~45 => exp fits fp32 fine);
    rowsum via an extra N=1 matmul vs a ones vector; normalize at the end
    with DVE reciprocal + per-partition scalar multiply.
  - float32r (1 PE cycle/row vs 4 for fp32) for the q/k path (logit
    precision); bf16 for the v path and P (post-softmax quantization only).
"""

import sys

if "/opt/trn_rl_repo" not in sys.path:
    sys.path.insert(0, "/opt/trn_rl_repo")

import numpy as np

from concourse import bacc, tile, mybir
import concourse.bass as bass
from concourse.bass_utils import run_bass_kernel_spmd

B, S, D, HD = 4, 2048, 1024, 64
N_CORES = 8
CPC = 512  # output channels per core
NCHUNK = 8  # d_model / 128

f32 = mybir.dt.float32
f32r = mybir.dt.float32r
bf16 = mybir.dt.bfloat16
AF = mybir.ActivationFunctionType
ALU = mybir.AluOpType

_cache = {}


def _build():
    nc = bacc.Bacc("TRN2", target_bir_lowering=False, debug=False, num_devices=N_CORES)

    xT = nc.dram_tensor("xT", [NCHUNK, 128, S], bf16, kind="ExternalInput")
    wqkT = nc.dram_tensor("wqkT", [NCHUNK, 128, 128], bf16, kind="ExternalInput")
    bqk = nc.dram_tensor("bqk", [128, 1], f32, kind="ExternalInput")
    wvT = nc.dram_tensor("wvT", [NCHUNK, 128, CPC], bf16, kind="ExternalInput")
    masks = nc.dram_tensor("masks", [4, 128, 512], bf16, kind="ExternalInput")
    out = nc.dram_tensor("out", [16, 128, CPC], f32, kind="ExternalOutput")

    with tile.TileContext(nc) as tc:
        with (
            tc.tile_pool(name="big", bufs=1) as big,
            tc.tile_pool(name="ppool", bufs=30) as ppool,
            tc.tile_pool(name="opool", bufs=3) as opool,
            tc.tile_pool(name="small", bufs=4) as small,
            tc.tile_pool(name="ps_s", bufs=3, space=bass.MemorySpace.PSUM) as ps_s,
            tc.tile_pool(name="ps_v", bufs=2, space=bass.MemorySpace.PSUM) as ps_v,
            tc.tile_pool(name="ps_o", bufs=2, space=bass.MemorySpace.PSUM) as ps_o,
            tc.tile_pool(name="ps_rb", bufs=1, space=bass.MemorySpace.PSUM) as ps_rb,
        ):
            # persistent SBUF tiles
            xt = big.tile([128, NCHUNK, S], bf16, tag="xt")  # 32KB/p
            wqk = big.tile([128, NCHUNK, 128], bf16, tag="wqk")  # 2KB/p
            wv = big.tile([128, NCHUNK, CPC], bf16, tag="wv")  # 8KB/p
            bqk_sb = big.tile([128, 1], f32, tag="bqk")
            mask_sb = big.tile([128, 4, 512], bf16, tag="mask")  # 4KB/p
            qk_sb = big.tile([128, S], f32r, tag="qk")  # 8KB/p
            kt_sb = big.tile([64, S], f32r, tag="kt")  # 8KB/p
            v_sb = big.tile([128, 16, CPC], bf16, tag="v")  # 16KB/p
            ones_b = big.tile([128, 128], bf16, tag="ones_b")

            # input DMAs, most-urgent first: wqk+xt feed the first matmuls
            for c in range(NCHUNK):
                nc.sync.dma_start(out=wqk[:, c, :], in_=wqkT[c, :, :])
                nc.sync.dma_start(out=xt[:, c, :], in_=xT[c, :, :])
            nc.sync.dma_start(out=bqk_sb[:, :], in_=bqk[:, :])
            for c in range(NCHUNK):
                nc.sync.dma_start(out=wv[:, c, :], in_=wvT[c, :, :])
            for m in range(4):
                nc.sync.dma_start(out=mask_sb[:, m, :], in_=masks[m, :, :])
            nc.vector.memset(ones_b[:, :], 1.0)

            # ---- Q/K projection: qkT [128h (64 q + 64 k), S] ----
            # chunk-outer (pairs of blocks) so compute starts on chunk 0;
            # accumulators share the scores pool's PSUM slots (same tag)
            for pair in range(2):
                qk_ps = [
                    ps_s.tile([128, 512], f32, tag="scps", name=f"qkps{pair}{jj}")
                    for jj in range(2)
                ]
                for c in range(NCHUNK):
                    for jj in range(2):
                        j = 2 * pair + jj
                        nc.tensor.matmul(
                            qk_ps[jj][:, :],
                            wqk[:, c, :],
                            xt[:, c, 512 * j : 512 * (j + 1)],
                            start=(c == 0),
                            stop=(c == NCHUNK - 1),
                        )
                for jj in range(2):
                    j = 2 * pair + jj
                    nc.scalar.activation(
                        qk_sb[:, 512 * j : 512 * (j + 1)],
                        qk_ps[jj][:, :],
                        AF.Identity,
                        bias=bqk_sb[:, 0:1],
                    )
                    # kT rows (64..128) -> partitions 0..64, per block
                    nc.sync.dma_start(
                        out=kt_sb[:, 512 * j : 512 * (j + 1)],
                        in_=qk_sb[64:128, 512 * j : 512 * (j + 1)],
                    )

            # ---- V projection (bf16; bias folded via K=1 ones matmul) ----
            for t in range(16):
                v_ps = ps_v.tile([128, CPC], f32, tag="vps")
                for c in range(NCHUNK):
                    nc.tensor.matmul(
                        v_ps[:, :],
                        xt[:, c, 128 * t : 128 * (t + 1)],
                        wv[:, c, :],
                        start=(c == 0),
                        stop=(c == NCHUNK - 1),
                    )
                nc.scalar.copy(v_sb[:, t, :], v_ps[:, :])

            # ---- attention: blocks in reverse order, software-pipelined ----
            # Section s runs block j's scores/exp/rowsum while the previous
            # section's (larger) block does its attn@V - interleaved in PE
            # program order so neither phase stalls the in-order PE queue.
            LAG = 2  # rb(i) emitted after sc(i+LAG) so exp(i) is done

            def emit_scores(j, i):
                sc_ps = ps_s.tile([128, 512], f32, tag="scps", name=f"scps{j}_{i}")
                nc.tensor.matmul(
                    sc_ps[:, :],
                    kt_sb[:, 128 * i : 128 * (i + 1)],
                    qk_sb[0:64, 512 * j : 512 * (j + 1)],
                    start=True,
                    stop=True,
                )
                p = ppool.tile([128, 512], bf16, tag="p", name=f"p{j}_{i}")
                nc.scalar.activation(p[:, :], sc_ps[:, :], AF.Exp)
                if i >= 4 * j:
                    nc.vector.tensor_tensor(
                        p[:, :], p[:, :], mask_sb[:, i - 4 * j, :], ALU.mult
                    )
                return p

            def emit_rb(j, rb_ps, P, i):
                # ones128.T @ P accumulates the key-sum per query, broadcast
                # to every partition (the layout the P-normalize needs)
                nc.tensor.matmul(
                    rb_ps[:, :],
                    ones_b[:, :],
                    P[i][:, :],
                    start=(i == 0),
                    stop=(i == 4 * j + 3),
                )

            def attnv_ops(j, P):
                ops = []
                for tq in range(4):
                    t = 4 * j + tq
                    ops.append(("alloc", t))
                    for i in range(t + 1):
                        ops.append(("mm", t, i))
                    ops.append(("evac", t))
                return ops

            def emit_attnv_op(op, P, state):
                if op[0] == "alloc":
                    t = op[1]
                    state[t] = ps_o.tile([128, CPC], f32, tag="ops", name=f"ops{t}")
                elif op[0] == "mm":
                    _, t, i = op
                    nc.tensor.matmul(
                        state[t][:, :],
                        P[i][:, 128 * (t % 4) : 128 * (t % 4) + 128],
                        v_sb[:, i, :],
                        start=(i == 0),
                        stop=(i == t),
                    )
                else:
                    t = op[1]
                    o_sb = opool.tile([128, CPC], f32, tag="osb", name=f"osb{t}")
                    nc.vector.tensor_copy(o_sb[:, :], state[t][:, :])
                    nc.sync.dma_start(out=out[t, :, :], in_=o_sb[:, :])

            prev = None  # (j, P_normalized) of the block awaiting attn@V
            for j in [3, 2, 1, 0, None]:
                av = attnv_ops(*prev) if prev is not None else []
                avP = prev[1] if prev is not None else None
                av_state = {}
                if j is None:
                    for op in av:
                        emit_attnv_op(op, avP, av_state)
                    break
                n = 4 * j + 4
                rb_ps = ps_rb.tile([128, 512], f32, tag="rbps", name=f"rbps{j}")
                P = []
                # A-sequence: scores with rb lagged behind by LAG
                A = []
                for i in range(n):
                    A.append(("sc", i))
                    if i >= LAG:
                        A.append(("rb", i - LAG))
                for i in range(max(0, n - LAG), n):
                    A.append(("rb", i))
                # front-load a few scores to cover the pipeline refill, then
                # interleave the previous block's attn@V ops
                front = min(4, len(A))
                k_av = 0
                for idx, aop in enumerate(A):
                    if aop[0] == "sc":
                        P.append(emit_scores(j, aop[1]))
                    else:
                        emit_rb(j, rb_ps, P, aop[1])
                    if idx >= front - 1:
                        want = (idx + 1 - front + 1) * len(av) / max(
                            1, len(A) - front + 1
                        )
                        while k_av < len(av) and k_av < want:
                            emit_attnv_op(av[k_av], avP, av_state)
                            k_av += 1
                while k_av < len(av):
                    emit_attnv_op(av[k_av], avP, av_state)
                    k_av += 1
                # fast reciprocal of the rowsum broadcast, downcast to bf16,
                # then prenormalize P (4x-mode bf16 multiplies)
                rcp = small.tile([128, 512], f32, tag="rcp", name=f"rcp{j}")
                nc.vector.reciprocal_approx_fast(rcp[:, :], rb_ps[:, :])
                rcp_bf = small.tile([128, 512], bf16, tag="rcpbf", name=f"rcpbf{j}")
                nc.vector.tensor_copy(rcp_bf[:, :], rcp[:, :])
                for i in range(n):
                    nc.vector.tensor_tensor(
                        P[i][:, :], P[i][:, :], rcp_bf[:, :], ALU.mult
                    )
                prev = (j, P)

    nc.compile()
    return nc


def _get_nc():
    if "nc" not in _cache:
        _cache["nc"] = _build()
    return _cache["nc"]


def _prep_in_maps(x, WQ_w, WQ_b, WK_w, WK_b, WV_w, WV_b):
    bf = mybir.dt.np(bf16)
    wqk = np.concatenate([WQ_w, WK_w], axis=0)  # [128, D]
    wqkT = np.ascontiguousarray(wqk.T.reshape(NCHUNK, 128, 128)).astype(bf)
    bqk = np.concatenate([WQ_b, WK_b]).reshape(128, 1).astype(np.float32, copy=False)

    # masks[m, kk, qq] = 1 if 128*m + kk <= qq else 0
    kk = np.arange(128)[:, None]
    qq = np.arange(512)[None, :]
    masks = np.stack([(128 * m + kk <= qq) for m in range(4)], axis=0).astype(bf)

    in_maps = []
    for core in range(N_CORES):
        b, half = core // 2, core % 2
        xTb = np.ascontiguousarray(x[b].T).reshape(NCHUNK, 128, S)
        wv_sl = WV_w[half * CPC : (half + 1) * CPC]  # [CPC, D]
        wvT = np.ascontiguousarray(wv_sl.T).reshape(NCHUNK, 128, CPC)
        in_maps.append(
            {
                "xT": xTb.astype(bf),
                "wqkT": wqkT,
                "bqk": bqk,
                "wvT": wvT.astype(bf),
                "masks": masks,
            }
        )
    return in_maps


def _run(in_maps, trace=False, **kw):
    nc = _get_nc()
    return run_bass_kernel_spmd(
        nc, in_maps, core_ids=list(range(N_CORES)), trace=trace, **kw
    )


def kernel(x, WQ_w, WQ_b, WK_w, WK_b, WV_w, WV_b):
    x = np.asarray(x, dtype=np.float32)
    in_maps = _prep_in_maps(
        x,
        np.asarray(WQ_w, np.float32),
        np.asarray(WQ_b, np.float32),
        np.asarray(WK_w, np.float32),
        np.asarray(WK_b, np.float32),
        np.asarray(WV_w, np.float32),
        np.asarray(WV_b, np.float32),
    )
    res = _run(in_maps, trace=False)
    out = np.empty((B, S, D), dtype=np.float32)
    for core in range(N_CORES):
        b, half = core // 2, core % 2
        shard = res.results[core]["out"].reshape(S, CPC)
        out[b, :, half * CPC : (half + 1) * CPC] = shard
    out += np.asarray(WV_b, np.float32)[None, None, :]
    return out


# revision 10
# speedup vs baseline: 1.4076x; 1.0589x over previous
"""Distributed Bass attention kernel for 8 TRN2 NeuronCores.

Problem: single-head causal attention, B=4, S=2048, d_model=1024, d_head=64.
  q = x@WQ.T+bq; k = x@WK.T+bk; v = x@WV.T+bv (v is d_model wide)
  out = softmax(causal(q@k.T)) @ v

Sharding: core = 2*b + half. Each core computes batch b, output channels
[half*512, (half+1)*512). Q/K/scores/softmax are duplicated within a batch
pair (cheap); V projection and attn@V are channel-split. No collectives.

Layout tricks:
  - x fed pre-transposed (xT [d, S]) so projections contract d on partitions.
  - scores computed transposed [keys, queries] (kT stationary, qT moving) so
    attn@V uses P tiles directly as the stationary operand - no transposes.
  - softmax without max-subtraction (|logits| <= ~45 => exp fits fp32 fine);
    rowsum via an extra N=1 matmul vs a ones vector; normalize at the end
    with DVE reciprocal + per-partition scalar multiply.
  - float32r (1 PE cycle/row vs 4 for fp32) for the q/k path (logit
    precision); bf16 for the v path and P (post-softmax quantization only).
"""

import sys

if "/opt/trn_rl_repo" not in sys.path:
    sys.path.insert(0, "/opt/trn_rl_repo")

import numpy as np

from concourse import bacc, tile, mybir
import concourse.bass as bass
from concourse.bass_utils import run_bass_kernel_spmd

B, S, D, HD = 4, 2048, 1024, 64
N_CORES = 8
CPC = 512  # output channels per core
NCHUNK = 8  # d_model / 128

f32 = mybir.dt.float32
f32r = mybir.dt.float32r
bf16 = mybir.dt.bfloat16
AF = mybir.ActivationFunctionType
ALU = mybir.AluOpType

_cache = {}


def _build():
    nc = bacc.Bacc("TRN2", target_bir_lowering=False, debug=False, num_devices=N_CORES)

    xT = nc.dram_tensor("xT", [NCHUNK, 128, S], bf16, kind="ExternalInput")
    wqkT = nc.dram_tensor("wqkT", [NCHUNK, 128, 128], bf16, kind="ExternalInput")
    bqk = nc.dram_tensor("bqk", [128, 1], f32, kind="ExternalInput")
    wvT = nc.dram_tensor("wvT", [NCHUNK, 128, CPC], bf16, kind="ExternalInput")
    masks = nc.dram_tensor("masks", [4, 128, 512], bf16, kind="ExternalInput")
    out = nc.dram_tensor("out", [16, 128, CPC], f32, kind="ExternalOutput")

    with tile.TileContext(nc) as tc:
        with (
            tc.tile_pool(name="big", bufs=1) as big,
            tc.tile_pool(name="ppool", bufs=30) as ppool,
            tc.tile_pool(name="opool", bufs=3) as opool,
            tc.tile_pool(name="small", bufs=4) as small,
            tc.tile_pool(name="ps_s", bufs=3, space=bass.MemorySpace.PSUM) as ps_s,
            tc.tile_pool(name="ps_v", bufs=2, space=bass.MemorySpace.PSUM) as ps_v,
            tc.tile_pool(name="ps_o", bufs=2, space=bass.MemorySpace.PSUM) as ps_o,
            tc.tile_pool(name="ps_rb", bufs=1, space=bass.MemorySpace.PSUM) as ps_rb,
        ):
            # persistent SBUF tiles
            xt = big.tile([128, NCHUNK, S], bf16, tag="xt")  # 32KB/p
            wqk = big.tile([128, NCHUNK, 128], bf16, tag="wqk")  # 2KB/p
            wv = big.tile([128, NCHUNK, CPC], bf16, tag="wv")  # 8KB/p
            bqk_sb = big.tile([128, 1], f32, tag="bqk")
            mask_sb = big.tile([128, 4, 512], bf16, tag="mask")  # 4KB/p
            qk_sb = big.tile([128, S], f32r, tag="qk")  # 8KB/p
            kt_sb = big.tile([64, S], f32r, tag="kt")  # 8KB/p
            v_sb = big.tile([128, 16, CPC], bf16, tag="v")  # 16KB/p
            ones_b = big.tile([128, 128], bf16, tag="ones_b")
            q2_sb = big.tile([128, S], f32r, tag="q2")  # rows 64-127 = q copy

            # input DMAs, most-urgent first: wqk+xt feed the first matmuls
            for c in range(NCHUNK):
                nc.sync.dma_start(out=wqk[:, c, :], in_=wqkT[c, :, :])
                nc.sync.dma_start(out=xt[:, c, :], in_=xT[c, :, :])
            nc.sync.dma_start(out=bqk_sb[:, :], in_=bqk[:, :])
            for c in range(NCHUNK):
                nc.sync.dma_start(out=wv[:, c, :], in_=wvT[c, :, :])
            for m in range(4):
                nc.sync.dma_start(out=mask_sb[:, m, :], in_=masks[m, :, :])
            nc.vector.memset(ones_b[:, :], 1.0)

            # ---- Q/K projection: qkT [128h (64 q + 64 k), S] ----
            # chunk-outer (pairs of blocks) so compute starts on chunk 0;
            # accumulators share the scores pool's PSUM slots (same tag)
            for pair in range(2):
                qk_ps = [
                    ps_s.tile([128, 512], f32, tag="scps", name=f"qkps{pair}{jj}")
                    for jj in range(2)
                ]
                for c in range(NCHUNK):
                    for jj in range(2):
                        j = 2 * pair + jj
                        nc.tensor.matmul(
                            qk_ps[jj][:, :],
                            wqk[:, c, :],
                            xt[:, c, 512 * j : 512 * (j + 1)],
                            start=(c == 0),
                            stop=(c == NCHUNK - 1),
                        )
                for jj in range(2):
                    j = 2 * pair + jj
                    nc.scalar.activation(
                        qk_sb[:, 512 * j : 512 * (j + 1)],
                        qk_ps[jj][:, :],
                        AF.Identity,
                        bias=bqk_sb[:, 0:1],
                    )
                    # kT rows (64..128) -> partitions 0..64, per block
                    nc.sync.dma_start(
                        out=kt_sb[:, 512 * j : 512 * (j + 1)],
                        in_=qk_sb[64:128, 512 * j : 512 * (j + 1)],
                    )
                    # q rows (0..64) -> partitions 64..128 (row-packed scores)
                    nc.sync.dma_start(
                        out=q2_sb[64:128, 512 * j : 512 * (j + 1)],
                        in_=qk_sb[0:64, 512 * j : 512 * (j + 1)],
                    )

            # ---- V projection (bf16; bias folded via K=1 ones matmul) ----
            for t in range(16):
                v_ps = ps_v.tile([128, CPC], f32, tag="vps")
                for c in range(NCHUNK):
                    nc.tensor.matmul(
                        v_ps[:, :],
                        xt[:, c, 128 * t : 128 * (t + 1)],
                        wv[:, c, :],
                        start=(c == 0),
                        stop=(c == NCHUNK - 1),
                    )
                nc.scalar.copy(v_sb[:, t, :], v_ps[:, :])

            # ---- attention: blocks in reverse order, software-pipelined ----
            # Section s runs block j's scores/exp/rowsum while the previous
            # section's (larger) block does its attn@V - interleaved in PE
            # program order so neither phase stalls the in-order PE queue.
            LAG = 2  # rb(i) emitted after sc(i+LAG) so exp(i) is done

            def emit_scores(j, i):
                # row-packed pair: key tile i on array rows 0-63, key tile
                # i+1 on rows 64-127 - the two matmuls run concurrently
                ps_pair = []
                for half, ii in ((0, i), (1, i + 1)):
                    sc_ps = ps_s.tile(
                        [128, 512], f32, tag="scps", name=f"scps{j}_{ii}"
                    )
                    if half == 0:
                        lhsT = kt_sb[:, 128 * ii : 128 * (ii + 1)]
                        rhs = qk_sb[0:64, 512 * j : 512 * (j + 1)]
                    else:
                        lhsT = qk_sb[64:128, 128 * ii : 128 * (ii + 1)]
                        rhs = q2_sb[64:128, 512 * j : 512 * (j + 1)]
                    nc.tensor.matmul(sc_ps[:, :], lhsT, rhs, start=True, stop=True)
                    ps_pair.append(sc_ps)
                out_pair = []
                for k, sc_ps in enumerate(ps_pair):
                    ii = i + k
                    p = ppool.tile([128, 512], bf16, tag="p", name=f"p{j}_{ii}")
                    nc.scalar.activation(p[:, :], sc_ps[:, :], AF.Exp)
                    if ii >= 4 * j:
                        nc.vector.tensor_tensor(
                            p[:, :], p[:, :], mask_sb[:, ii - 4 * j, :], ALU.mult
                        )
                    out_pair.append(p)
                return out_pair

            def emit_rb(j, rb_ps, P, i):
                # ones128.T @ P accumulates the key-sum per query, broadcast
                # to every partition (the layout the P-normalize needs)
                nc.tensor.matmul(
                    rb_ps[:, :],
                    ones_b[:, :],
                    P[i][:, :],
                    start=(i == 0),
                    stop=(i == 4 * j + 3),
                )

            def attnv_ops(j, P):
                ops = []
                for tq in range(4):
                    t = 4 * j + tq
                    ops.append(("alloc", t))
                    for i in range(t + 1):
                        ops.append(("mm", t, i))
                    ops.append(("evac", t))
                return ops

            def emit_attnv_op(op, P, state):
                if op[0] == "alloc":
                    t = op[1]
                    state[t] = ps_o.tile([128, CPC], f32, tag="ops", name=f"ops{t}")
                elif op[0] == "mm":
                    _, t, i = op
                    nc.tensor.matmul(
                        state[t][:, :],
                        P[i][:, 128 * (t % 4) : 128 * (t % 4) + 128],
                        v_sb[:, i, :],
                        start=(i == 0),
                        stop=(i == t),
                    )
                else:
                    t = op[1]
                    o_sb = opool.tile([128, CPC], f32, tag="osb", name=f"osb{t}")
                    nc.vector.tensor_copy(o_sb[:, :], state[t][:, :])
                    nc.sync.dma_start(out=out[t, :, :], in_=o_sb[:, :])

            prev = None  # (j, P_normalized) of the block awaiting attn@V
            for j in [3, 2, 1, 0, None]:
                av = attnv_ops(*prev) if prev is not None else []
                avP = prev[1] if prev is not None else None
                av_state = {}
                if j is None:
                    for op in av:
                        emit_attnv_op(op, avP, av_state)
                    break
                n = 4 * j + 4
                rb_ps = ps_rb.tile([128, 512], f32, tag="rbps", name=f"rbps{j}")
                P = []
                # A-sequence: score pairs with rb lagged one pair behind
                A = []
                for i in range(0, n, 2):
                    A.append(("sc", i))
                    if i >= 2:
                        A.append(("rb", i - 2))
                        A.append(("rb", i - 1))
                A.append(("rb", n - 2))
                A.append(("rb", n - 1))
                # front-load a few scores to cover the pipeline refill, then
                # interleave the previous block's attn@V ops
                front = min(4, len(A))
                k_av = 0
                for idx, aop in enumerate(A):
                    if aop[0] == "sc":
                        P.extend(emit_scores(j, aop[1]))
                    else:
                        emit_rb(j, rb_ps, P, aop[1])
                    if idx >= front - 1:
                        want = (idx + 1 - front + 1) * len(av) / max(
                            1, len(A) - front + 1
                        )
                        while k_av < len(av) and k_av < want:
                            emit_attnv_op(av[k_av], avP, av_state)
                            k_av += 1
                while k_av < len(av):
                    emit_attnv_op(av[k_av], avP, av_state)
                    k_av += 1
                # fast reciprocal of the rowsum broadcast, downcast to bf16,
                # then prenormalize P (4x-mode bf16 multiplies)
                rcp = small.tile([128, 512], f32, tag="rcp", name=f"rcp{j}")
                nc.vector.reciprocal_approx_fast(rcp[:, :], rb_ps[:, :])
                rcp_bf = small.tile([128, 512], bf16, tag="rcpbf", name=f"rcpbf{j}")
                nc.vector.tensor_copy(rcp_bf[:, :], rcp[:, :])
                for i in range(n):
                    nc.vector.tensor_tensor(
                        P[i][:, :], P[i][:, :], rcp_bf[:, :], ALU.mult
                    )
                prev = (j, P)

    nc.compile()
    return nc


def _get_nc():
    if "nc" not in _cache:
        _cache["nc"] = _build()
    return _cache["nc"]


def _prep_in_maps(x, WQ_w, WQ_b, WK_w, WK_b, WV_w, WV_b):
    bf = mybir.dt.np(bf16)
    wqk = np.concatenate([WQ_w, WK_w], axis=0)  # [128, D]
    wqkT = np.ascontiguousarray(wqk.T.reshape(NCHUNK, 128, 128)).astype(bf)
    bqk = np.concatenate([WQ_b, WK_b]).reshape(128, 1).astype(np.float32, copy=False)

    # masks[m, kk, qq] = 1 if 128*m + kk <= qq else 0
    kk = np.arange(128)[:, None]
    qq = np.arange(512)[None, :]
    masks = np.stack([(128 * m + kk <= qq) for m in range(4)], axis=0).astype(bf)

    in_maps = []
    for core in range(N_CORES):
        b, half = core // 2, core % 2
        xTb = np.ascontiguousarray(x[b].T).reshape(NCHUNK, 128, S)
        wv_sl = WV_w[half * CPC : (half + 1) * CPC]  # [CPC, D]
        wvT = np.ascontiguousarray(wv_sl.T).reshape(NCHUNK, 128, CPC)
        in_maps.append(
            {
                "xT": xTb.astype(bf),
                "wqkT": wqkT,
                "bqk": bqk,
                "wvT": wvT.astype(bf),
                "masks": masks,
            }
        )
    return in_maps


def _run(in_maps, trace=False, **kw):
    nc = _get_nc()
    return run_bass_kernel_spmd(
        nc, in_maps, core_ids=list(range(N_CORES)), trace=trace, **kw
    )


def kernel(x, WQ_w, WQ_b, WK_w, WK_b, WV_w, WV_b):
    x = np.asarray(x, dtype=np.float32)
    in_maps = _prep_in_maps(
        x,
        np.asarray(WQ_w, np.float32),
        np.asarray(WQ_b, np.float32),
        np.asarray(WK_w, np.float32),
        np.asarray(WK_b, np.float32),
        np.asarray(WV_w, np.float32),
        np.asarray(WV_b, np.float32),
    )
    res = _run(in_maps, trace=False)
    out = np.empty((B, S, D), dtype=np.float32)
    for core in range(N_CORES):
        b, half = core // 2, core % 2
        shard = res.results[core]["out"].reshape(S, CPC)
        out[b, :, half * CPC : (half + 1) * CPC] = shard
    out += np.asarray(WV_b, np.float32)[None, None, :]
    return out
